# revision 1
# baseline (speedup 1.0000x reference)
"""VQ codebook assignment kernel for Trainium2 (8 NeuronCores).

Problem: X (8,4096,128) f32, centroids (1024,128), mean/scale (128,),
mask (8,4096). Output: one-hot C (8,4096,1024) f32 of the nearest
centroid (L2 over standardized points), times mask.

Strategy (data-parallel, core b owns batch b):
  argmin_k ||(x-mean)/scale - c_k||^2 == argmax_k [ x . (c_k/scale) - b_k ]
  with b_k = mean . (c_k/scale) + ||c_k||^2 / 2.
  Scores are computed on the PE with a 3-term fp16 split matmul
  (Xh@Ch + Xh@Cl + Xl@Ch, fp32 PSUM accumulation); fp16 products are
  exact in f32, so every argmax matches the f32 reference.
  The -b_k bias enters the same PSUM accumulation via a rank-3 fp16
  matmul (ones3 x 3-way fp16 split of -b).
  One-hot extraction: m = rowmax(scores) on DVE, then on ACT
  out = Exp(scores * 2^100 - m * 2^100): the scale is a power of two so
  the products are exact; the result is exactly 1.0 at the argmax and
  exactly 0.0 elsewhere (Exp underflows). Zero mask rows are handled by
  adding ln(mask) (= -inf) to the bias.
"""
import numpy as np

import concourse.bass as bass
import concourse.bacc as bacc
import concourse.mybir as mybir
import concourse.tile as tile
from concourse import masks
from concourse.bass_utils import run_bass_kernel_spmd

B, N, D, K = 8, 4096, 128, 1024
PT = 128           # points per tile
NT = N // PT       # tiles per core
NC_ = K // 128     # centroid chunks
F32 = mybir.dt.float32
F16 = mybir.dt.float16
AF = mybir.ActivationFunctionType
OP = mybir.AluOpType
BIG = 2.0 ** 100


def _body(nc, tc, x_in, mask_in, c_in, mean_in, scale_in, out):
    import contextlib
    with contextlib.ExitStack() as ctx:
        # PSUM map: ps_s = 3 x [128,1024] score slots (6 banks);
        # ps_x = 2 x [128,512] slots (2 banks) shared by setup transposes,
        # the bias matmuls and 4-packed X transposes.
        ps_s = ctx.enter_context(tc.tile_pool(name="ps_s", bufs=2, space="PSUM"))
        ps_x = ctx.enter_context(tc.tile_pool(name="ps_x", bufs=2, space="PSUM"))

        sb = ctx.enter_context(tc.tile_pool(name="setup_sb", bufs=1))
        const = ctx.enter_context(tc.tile_pool(name="const", bufs=1))
        xin_pool = ctx.enter_context(tc.tile_pool(name="xin", bufs=8))
        xs_pool = ctx.enter_context(tc.tile_pool(name="xs", bufs=12))
        mcol_pool = ctx.enter_context(tc.tile_pool(name="mcol", bufs=4))
        oh_pool = ctx.enter_context(tc.tile_pool(name="oh", bufs=4))

        # centroids first: their DMA latency is on the setup critical path
        ck_all = sb.tile([128, K], F32)
        nc.sync.dma_start(ck_all[:].rearrange("p (t d) -> p t d", d=D),
                          c_in[:].rearrange("(t p) d -> p t d", p=128))

        ident = const.tile([128, 128], F32)
        masks.make_identity(nc, ident[:])
        identh = const.tile([128, 128], F16)
        masks.make_identity(nc, identh[:])

        ms = sb.tile([2, 128], F32)
        nc.sync.dma_start(ms[0:1, :], mean_in[:].unsqueeze(0))
        nc.sync.dma_start(ms[1:2, :], scale_in[:].unsqueeze(0))
        maskrow = sb.tile([NT, 128], F32)
        nc.sync.dma_start(maskrow[:], mask_in[:].rearrange("(t p) -> t p", t=NT))

        msT = const.tile([128, 2], F32)
        lnmask = const.tile([128, NT], F32)
        centT = sb.tile([128, K], F32)     # raw centroids^T [d, k]
        negb3 = const.tile([3, K], F16)

        PRE = 0  # tiles fully prepped (DMA+transpose+fp16 split) before bias

        xt4 = {}   # group -> [128,512] psum tile holding 4 transposed x tiles
        xhs, xls = {}, {}

        def x_fetch(t):
            x_raw = xin_pool.tile([PT, 128], F32)
            nc.sync.dma_start(x_raw[:], x_in[bass.ts(t, PT), :])
            return x_raw

        def x_transpose(t, x_raw):
            xt = ps_x.tile([128, PT], F32, tag="xt", name=f"xt_{t}")
            nc.tensor.transpose(xt[:], x_raw[:], ident[:])
            return xt[:]

        def x_hi(t, xt_sl):
            xh = xs_pool.tile([128, PT], F16, tag="xh")
            nc.scalar.activation(xh[:], xt_sl, AF.Copy)
            xhs[t] = xh
            return xh

        def x_lo(t, xt_sl, xh):
            xl = xs_pool.tile([128, PT], F16, tag="xl")
            nc.vector.tensor_tensor(xl[:], xt_sl, xh[:], op=OP.subtract)
            xls[t] = xl
            return xl

        pss = ctx.enter_context(tc.tile_pool(name="setup_ps", bufs=2, space="PSUM"))
        if True:
            p_mk = pss.tile([128, 128], F32, tag="ct")
            nc.tensor.transpose(p_mk[:, 0:NT], maskrow[:], ident[0:NT, 0:NT])
            # ln(mask): 0 for mask==1, -inf for mask==0
            nc.scalar.activation(lnmask[:], p_mk[:, 0:NT], AF.Ln)

            p_ms = pss.tile([128, 128], F32, tag="ct")
            nc.tensor.transpose(p_ms[:, 0:2], ms[:], ident[0:2, 0:2])
            nc.scalar.activation(msT[:], p_ms[:, 0:2], AF.Copy)

            for t in range(NC_):
                p_ct = pss.tile([128, 128], F32, tag="ct")
                nc.tensor.transpose(p_ct[:], ck_all[:, bass.ts(t, 128)], ident[:])
                nc.scalar.activation(centT[:, bass.ts(t, 128)], p_ct[:], AF.Copy)

            # PE filler while the DVE/ACT crunch cp/csq/ch/cl: fully prep the
            # first PRE tiles' transposes and fp16 hi-parts.
            pre_slices = {}
            for t in range(PRE):
                x_raw = x_fetch(t)
                sl = x_transpose(t, x_raw)
                pre_slices[t] = sl
                x_hi(t, sl)

            recip = const.tile([128, 1], F32)
            nc.vector.reciprocal(recip[:], msT[:, 1:2])

            # c' = c/scale (f32), squared raw c, fp16 split of c'
            cp = sb.tile([128, K], F32)
            nc.vector.tensor_scalar(cp[:], centT[:], recip[:], None, op0=OP.mult)
            csq = sb.tile([128, K], F32)
            nc.vector.tensor_tensor(csq[:], centT[:], centT[:], op=OP.mult)
            ch = const.tile([128, K], F16)
            nc.scalar.activation(ch[:], cp[:], AF.Copy)
            cl = const.tile([128, K], F16)
            nc.vector.tensor_tensor(cl[:], cp[:], ch[:], op=OP.subtract)

            for t in range(PRE):
                x_lo(t, pre_slices[t], xhs[t])

            halfcol = sb.tile([128, 1], F32)
            nc.vector.memset(halfcol[:], 0.5)
            ones3 = const.tile([3, 128], F16)
            nc.vector.memset(ones3[:], 1.0)

            # bias in chunked layout [128 (k%128), 8 (k//128)]: per chunk two
            # N=1 f32 matmuls: cp_chunk.T @ mean  +  csq_chunk.T @ 0.5
            biasp = pss.tile([128, 128], F32, tag="ct")
            for t in range(NC_):
                nc.tensor.matmul(biasp[:, t:t + 1], cp[:, bass.ts(t, 128)],
                                 msT[:, 0:1], start=True, stop=False)
                nc.tensor.matmul(biasp[:, t:t + 1], csq[:, bass.ts(t, 128)],
                                 halfcol[:], start=False, stop=True)

            # fp16 3-way split of -bias, all in the parallel [128, 8] layout
            nb = sb.tile([128, NC_], F32)
            nc.scalar.activation(nb[:], biasp[:, 0:NC_], AF.Copy, scale=-1.0)
            bsplit = []
            r = nb
            for i in range(3):
                bi = sb.tile([128, NC_], F16, tag=f"b{i}")
                nc.vector.tensor_copy(bi[:], r[:])
                bsplit.append(bi)
                if i < 2:
                    r2 = sb.tile([128, NC_], F32, tag=f"r{i}")
                    nc.vector.tensor_tensor(r2[:], r[:], bi[:], op=OP.subtract)
                    r = r2
            # rows of negb3 = transpose of each split -> [8,128] -> one DMA
            for i, bi in enumerate(bsplit):
                p_bt = pss.tile([128, 128], F16, tag="ct")
                nc.tensor.transpose(p_bt[0:NC_, :], bi[:], identh[:])
                biT = sb.tile([NC_, 128], F16, tag=f"bT{i}")
                nc.vector.tensor_copy(biT[:], p_bt[0:NC_, :])
                nc.sync.dma_start(negb3[i:i + 1, :], biT[:])

        # ---- main loop ----
        for t in range(NT):
            if t < PRE:
                xh, xl = xhs[t], xls[t]
            else:
                x_raw = x_fetch(t)
                sl = x_transpose(t, x_raw)
                xh = x_hi(t, sl)
                xl = x_lo(t, sl, xh)

            sc = ps_s.tile([PT, K], F32)
            s0, s1 = (slice(0, 512), slice(512, 1024))
            nc.tensor.matmul(sc[:, s0], xh[:], ch[:, s0], start=True, stop=False)
            nc.tensor.matmul(sc[:, s1], xh[:], ch[:, s1], start=True, stop=False)
            nc.tensor.matmul(sc[:, s0], xh[:], cl[:, s0], start=False, stop=False)
            nc.tensor.matmul(sc[:, s1], xh[:], cl[:, s1], start=False, stop=False)
            nc.tensor.matmul(sc[:, s0], xl[:], ch[:, s0], start=False, stop=False)
            nc.tensor.matmul(sc[:, s1], xl[:], ch[:, s1], start=False, stop=False)
            nc.tensor.matmul(sc[:, s0], ones3[:], negb3[:, s0],
                             start=False, stop=True)
            nc.tensor.matmul(sc[:, s1], ones3[:], negb3[:, s1],
                             start=False, stop=True)

            m = mcol_pool.tile([PT, 1], F32, tag="m")
            nc.vector.reduce_max(m[:], sc[:], axis=mybir.AxisListType.X)
            bias_col = mcol_pool.tile([PT, 1], F32, tag="bias")
            nc.vector.tensor_scalar(bias_col[:], m[:], -BIG, lnmask[:, t:t + 1],
                                    op0=OP.mult, op1=OP.add)

            oh = oh_pool.tile([PT, K], F32)
            nc.scalar.activation(oh[:], sc[:], AF.Exp, bias=bias_col[:],
                                 scale=BIG)
            nc.gpsimd.dma_start(out[bass.ts(t, PT), :], oh[:])


def _build():
    # Bacc (not raw Bass): its compile() moves matmul waits onto ldweights and
    # splits oversized wait lists into event-semaphore instructions — without
    # it walrus rejects Tile output with "Too many sync wait commands".
    nc = bacc.Bacc("TRN2", target_bir_lowering=False, debug=False, num_devices=B)
    x_in = nc.dram_tensor("x", [N, D], F32, kind="ExternalInput")
    mask_in = nc.dram_tensor("mask", [N], F32, kind="ExternalInput")
    c_in = nc.dram_tensor("cent", [K, D], F32, kind="ExternalInput")
    mean_in = nc.dram_tensor("mean", [D], F32, kind="ExternalInput")
    scale_in = nc.dram_tensor("scale", [D], F32, kind="ExternalInput")
    out = nc.dram_tensor("out", [N, K], F32, kind="ExternalOutput")
    with tile.TileContext(nc) as tc:
        _body(nc, tc, x_in[:], mask_in[:], c_in[:], mean_in[:], scale_in[:], out[:])
    nc.compile()
    return nc


_NC = None


def _run(inputs, trace=False, tmpdir=None):
    global _NC
    if _NC is None:
        _NC = _build()
    X = np.ascontiguousarray(inputs["X"], dtype=np.float32)
    mask = np.ascontiguousarray(inputs["mask"], dtype=np.float32)
    cent = np.ascontiguousarray(inputs["centroids"], dtype=np.float32)
    mean = np.ascontiguousarray(inputs["mean"], dtype=np.float32)
    scale = np.ascontiguousarray(inputs["scale"], dtype=np.float32)
    in_maps = [
        {"x": X[b], "mask": mask[b], "cent": cent, "mean": mean, "scale": scale}
        for b in range(B)
    ]
    res = run_bass_kernel_spmd(_NC, in_maps, list(range(B)), trace=trace,
                               tmpdir=tmpdir,
                               trace_cores=[0] if trace else None)
    full = np.stack([res.results[b]["out"] for b in range(B)], axis=0)
    return full, res


def kernel(**inputs) -> np.ndarray:
    full, _ = _run(inputs, trace=False)
    return full

